# revision 13
# baseline (speedup 1.0000x reference)
"""Weighted-MAE loss (nn_MAELoss) on 8 Trainium2 NeuronCores.

reference:  w = bucket-weights(y_true) via thresholds log1p(5/25/50),
            loss = sum(w * |y_true - y_pred|) / sum(w)

Strategy: data-parallel over the batch dim (8 shards of 8 batches); each
core reduces its [128, 15360] shard to per-partition fp32 accumulators;
the host combines them in float64 and divides.

Per-core dataflow (~43.7us DMA floor for 15.7MB/core at 360 GB/s):
  DMA : yt/yp resident in SBUF, streamed in column chunks (yt_c then
        yp_c per chunk; head chunks sized >=480 so the HWDGE 625ns/DMA
        issue rate never starves the DMA engines; tail chunks small to
        shorten the post-stream dependency chain).
  DVE : E1_c = sum(((yt>=T1) + 0.2/29.8) * |yt-yp|)   (diff fused in)
        E2_c = sum(((yt>=T2) + r*(yt>=T3)) * |d|), d from GPSIMD
        tail chunks (R set) avoid the GPSIMD chain entirely:
        U2_c/U3_c = sum((yt>=T2/3) * |yt-yp|) fused ops, plus exact
        is_ge counts (tensor_scalar, 2x perf mode) for late columns.
  ACT : sign-counts for T1/T2/T3 on early columns (bias one ulp below
        each threshold so exact hits count as >=, like the reference).
  Pool: d = yt - yp for mid-stream chunks only (its ~2ns/col rate and
        the +900ns DMA-sem latency make it unusable near stream end).
All engine busy totals sit ~4us under the 43.7us DMA stream so the end
is bounded by the stream + a short tail; host combines in float64.
"""

import os
import sys

import numpy as np

try:
    import concourse  # noqa: F401
except ImportError:  # pragma: no cover
    for _p in ("/root/.axon_site/_ro/trn_rl_repo", "/opt/trn_rl_repo"):
        if os.path.isdir(_p) and _p not in sys.path:
            sys.path.append(_p)

from contextlib import ExitStack
from operator import add

import concourse.bacc as bacc
import concourse.tile as tile
from concourse import mybir
from concourse.bass_utils import run_bass_kernel_spmd
import concourse.dve_ops as dve_ops
from concourse.dve_ops import DveOp
from concourse.dve_spec import (
    C0,
    C1,
    C2,
    Spec,
    Src0,
    Src1,
    Zero,
    _has_src1,
    lower,
    maxx,
)
from concourse.dve_uop import DveOpSpec

# ----------------------------------------------------------------- problem
N_CORES = 8
B, C, T, H, W = 64, 1, 15, 128, 128
SHARD_B = B // N_CORES
P = 128
F = SHARD_B * C * T * H * W // P  # 15360
N_TOTAL = B * C * T * H * W      # 15728640

THR1 = float(np.float32(np.log1p(5.0)))
THR2 = float(np.float32(np.log1p(25.0)))
THR3 = float(np.float32(np.log1p(50.0)))
THRS = (THR1, THR2, THR3)
W_BASE = 0.2          # bucket-0 weight
DW1 = 29.8            # 30 - 0.2
DW2 = 2470.0          # 2500 - 30
DW3 = 17500.0         # 20000 - 2500
LAM1 = float(np.float32(W_BASE / DW1))   # folds 0.2*sum|d| into E1
RATIO32 = float(np.float32(DW3 / DW2))   # folds the T3 level into E2

# ------------------------------------------------------------ span layout
# DMA chunks (shared grid for yt/yp, issued yt_c then yp_c). Tile rotates
# HWDGE DMAs over an 8-slot sem ring: DMA k's issue waits for DMA k-8's
# completion sem (+900ns), so any 7 consecutive transfers must cover the
# ~2.2us issue+DGE path or the stream gaps -> tail chunks taper but stay
# >=256 cols; only the final chunk (fused, post-stream) is smaller.
CHUNKS = [448, 896, 1472] + [1920] * 5 + [768, 608, 480, 384, 320, 256, 128]
assert sum(CHUNKS) == F
NCH = len(CHUNKS)
CUM = [0]
for c in CHUNKS:
    CUM.append(CUM[-1] + c)

# Chunks whose E2-part uses the fused U2/U3 ops (no GPSIMD d): the chunks
# arriving too close to stream end for the Pool sub -> MASK2 chain.
R_SET = {NCH - 1}
POOL_SET = [ci for ci in range(NCH) if ci not in R_SET]

def _rdy(end):
    for i in range(NCH):
        if CUM[i + 1] >= end:
            return i
    raise AssertionError

# Engine rate budget per arriving column is ~2.84ns (360 GB/s, 8B/col);
# every engine's assigned rate must stay below it in each region or the
# deficit spills past the end of the DMA stream:
#   DVE: M1L+M2 2.08 + C3-at-2x 0.52 = 2.60
#   ACT: C1+C2 1.67 + ~0.4 per-op   = 2.1
#   Pool: sub 1.98
# C1/C2 sign spans (ACT, 2 passes): chunk-group spans (cheap per-op
# early); the last CC_TAIL chunks go to DVE tensor_scalar at 2x (ACT's
# 372ns/op fixed cost is too slow once data lands at stream end).
# C3: one early span on ACT (balance knob), per-chunk DVE 2x after.
CC_TAIL = 3          # final chunks whose C1/C2 ride DVE
C3_SPLIT_CH = 2      # C3 on ACT through this chunk index (inclusive)
ACT_GROUPS = [(0, 0), (1, 1), (2, 2), (3, 4), (5, 6),
              (7, 7), (8, 9), (10, 11)]

def _ck(i, j):
    return (CUM[i], CUM[j + 1])

SA = [_ck(i, j) for i, j in ACT_GROUPS if j < NCH - CC_TAIL]
assert SA[-1][1] == CUM[NCH - CC_TAIL]
SA3 = [(0, CUM[C3_SPLIT_CH + 1])]
SD3 = [_ck(i, i) for i in range(C3_SPLIT_CH + 1, NCH)]
SDCC = [_ck(i, i) for i in range(NCH - CC_TAIL, NCH)]

SA_RDY = [_rdy(b) for _, b in SA]
SA3_RDY = [_rdy(b) for _, b in SA3]
SD3_RDY = [_rdy(b) for _, b in SD3]
SDCC_RDY = [_rdy(b) for _, b in SDCC]

# accumulator slot layout in the [P, 128] acc tile
NE1 = NCH                      # E1 per chunk: slots 0..NCH-1
E2_SLOT = {ci: NE1 + k for k, ci in enumerate(POOL_SET)}
NU_BASE = NE1 + len(POOL_SET)
U_SLOT = {ci: NU_BASE + 2 * k for k, ci in enumerate(sorted(R_SET))}
ND3_BASE = NU_BASE + 2 * len(R_SET)      # DVE C3: + j
NDCC_BASE = ND3_BASE + len(SD3)          # DVE C1/C2 tails: + 2*j + k
NACC_BASE = NDCC_BASE + 2 * len(SDCC)    # ACT C1/C2: + 2*g + k
NA3_BASE = NACC_BASE + 2 * len(SA)       # ACT C3: + g
NSLOT = NA3_BASE + len(SA3)
ACC_W = 128
assert NSLOT <= ACC_W

# ------------------------------------------------------- custom DVE ops
_absdiff = maxx(Src0 - Src1, Src1 - Src0)  # |in0 - in1|
_neg = Zero - Src1
_absd = maxx(Src1, _neg)                   # |in1|  (in1 = precomputed d)


def _accum_ref(body_fn):
    def _r(in0, in1, s0, s1, imm2):
        b = body_fn(
            in0.astype(np.float32), None if in1 is None else in1.astype(np.float32),
            s0, s1, imm2,
        ).astype(np.float32)
        return b, b.reshape(b.shape[0], -1).sum(axis=-1, keepdims=True).astype(np.float32)
    return _r


def _register_op(name: str, spec: Spec) -> DveOp:
    for op in dve_ops.OPS:
        if op.name == name:
            return op
    row = dve_ops._CUSTOM_DVE_ROW_BASE + len(dve_ops.OPS)
    assert row < 0x20, "custom-DVE row overflow"
    shas = {}
    for ver in ("v3", "v4"):
        try:
            tmp = DveOpSpec(
                name=name, opcode=row, uops=lower(spec, ver=ver),
                rd1_en=_has_src1(spec),
            )
            shas[ver] = tmp.sha(ver)
        except Exception:
            pass
    op = DveOp(name, spec, subdim=False, uops_sha=shas)
    dve_ops.OPS.append(op)
    dve_ops._SUB_OPCODE_FOR_NAME[name] = row
    dve_ops.CUSTOM_DVE_SPECS[name] = spec
    return op


# out = ((in0 >= s0) + s1) * |in0 - in1| ; accum_out = sum(out)
MASK1L = _register_op(
    "WMAE_MASK1LD_ANT",
    Spec(body=((Src0 >= C0) + C1) * _absdiff, accum=add, accum_init=Zero,
         reference=_accum_ref(
             lambda a, b, s0, s1, i2: ((a >= s0) + s1) * np.abs(a - b))),
)
# out = ((in0 >= s0) + imm2*(in0 >= s1)) * |in1| ; accum_out = sum(out)
MASK2 = _register_op(
    "WMAE_MASK2_ANT",
    Spec(body=((Src0 >= C0) + C2 * (Src0 >= C1)) * _absd,
         accum=add, accum_init=Zero,
         reference=_accum_ref(
             lambda a, b, s0, s1, i2: ((a >= s0) + i2 * (a >= s1)) * np.abs(b))),
)
# out = (in0 >= s0) * |in0 - in1| ; accum_out = sum(out)  (tail chunks)
UABS = _register_op(
    "WMAE_UABS_ANT",
    Spec(body=(Src0 >= C0) * _absdiff, accum=add, accum_init=Zero,
         reference=_accum_ref(
             lambda a, b, s0, s1, i2: (a >= s0) * np.abs(a - b))),
)

_STATE: dict = {}


def _build():
    """Build + schedule the Bass module once per process."""
    if "nc" in _STATE:
        return _STATE["nc"]
    f32 = mybir.dt.float32
    nc = bacc.Bacc("TRN2", target_bir_lowering=False, debug=False,
                   enable_asserts=False)
    yt_d = nc.dram_tensor("y_true", [P, F], f32, kind="ExternalInput").ap()
    yp_d = nc.dram_tensor("y_pred", [P, F], f32, kind="ExternalInput").ap()
    out_d = nc.dram_tensor("partials", [P, ACC_W], f32,
                           kind="ExternalOutput").ap()

    with tile.TileContext(nc) as tc, ExitStack() as ctx:
        big_pool = ctx.enter_context(tc.tile_pool(name="big", bufs=1))
        mid_pool = ctx.enter_context(tc.tile_pool(name="mid", bufs=3))
        sm_pool = ctx.enter_context(tc.tile_pool(name="sm", bufs=6))
        junk_pool = ctx.enter_context(tc.tile_pool(name="junk", bufs=1))
        acc_pool = ctx.enter_context(tc.tile_pool(name="acc", bufs=1))

        yt = big_pool.tile([P, F], f32, tag="yt")
        yp = big_pool.tile([P, F], f32, tag="yp")
        acc = acc_pool.tile([P, ACC_W], f32, tag="acc")

        # sign(y + bias) counts y >= THR; bias = -(one ulp below THR) so an
        # exact threshold hit counts high, matching the reference's y < THR
        def _below(t):
            return float(np.nextafter(np.float32(t), np.float32(0.0)))

        biases = []
        for k, t in enumerate(THRS):
            bt = acc_pool.tile([P, 1], f32, tag=f"bias{k}")
            nc.vector.memset(bt[:], -_below(t))
            biases.append(bt)

        FS_MAX = max(CHUNKS)
        j_m1 = junk_pool.tile([P, FS_MAX], f32, tag="j_m1")
        j_m2 = junk_pool.tile([P, FS_MAX], f32, tag="j_m2")
        j_u = junk_pool.tile([P, 256], f32, tag="j_u")
        DW_MAX = max(b - a for a, b in SD3 + SDCC)
        AW_MAX = max(b - a for a, b in SA + SA3)
        j_cd = junk_pool.tile([P, DW_MAX], f32, tag="j_cd")
        j_ca = junk_pool.tile([P, AW_MAX], f32, tag="j_ca")

        # 1-element dummy Sign pulls the ACT table load into the DMA fill
        nc.scalar.activation(j_u[:, 0:1], biases[0][:],
                             mybir.ActivationFunctionType.Sign,
                             bias=biases[1][:])

        d_tiles: dict = {}

        def emit_m2(ci):
            a, b = CUM[ci], CUM[ci + 1]
            s = E2_SLOT[ci]
            nc.vector._custom_dve(
                MASK2, out=j_m2[:, :b - a], in0=yt[:, a:b],
                in1=d_tiles.pop(ci)[:, :b - a],
                s0=THR2, s1=THR3, imm2=RATIO32,
                accum_out=acc[:, s:s + 1],
            )

        for ci in range(NCH):
            a, b = CUM[ci], CUM[ci + 1]
            fs = b - a
            nc.sync.dma_start(yt[:, a:b], yt_d[:, a:b])
            # E2 for the previous chunk: its GPSIMD d completed during this
            # chunk's DMA window, so it is ready at the DVE queue head.
            if ci - 1 in E2_SLOT:
                emit_m2(ci - 1)
            for g, (sa, sb) in enumerate(SA):
                if SA_RDY[g] == ci:
                    for t in range(2):
                        q = NACC_BASE + 2 * g + t
                        nc.scalar.activation(
                            j_ca[:, :sb - sa], yt[:, sa:sb],
                            mybir.ActivationFunctionType.Sign,
                            bias=biases[t][:],
                            accum_out=acc[:, q:q + 1],
                        )
            for g, (sa, sb) in enumerate(SA3):
                if SA3_RDY[g] == ci:
                    q = NA3_BASE + g
                    nc.scalar.activation(
                        j_ca[:, :sb - sa], yt[:, sa:sb],
                        mybir.ActivationFunctionType.Sign,
                        bias=biases[2][:],
                        accum_out=acc[:, q:q + 1],
                    )
            for j, (sa, sb) in enumerate(SD3):
                if SD3_RDY[j] == ci:
                    q = ND3_BASE + j
                    nc.vector.tensor_scalar(
                        j_cd[:, :sb - sa], yt[:, sa:sb], THR3, 0.0,
                        mybir.AluOpType.is_ge, mybir.AluOpType.add,
                        accum_out=acc[:, q:q + 1],
                    )
            for j, (sa, sb) in enumerate(SDCC):
                if SDCC_RDY[j] == ci:
                    for t in range(2):
                        q = NDCC_BASE + 2 * j + t
                        nc.vector.tensor_scalar(
                            j_cd[:, :sb - sa], yt[:, sa:sb], THRS[t], 0.0,
                            mybir.AluOpType.is_ge, mybir.AluOpType.add,
                            accum_out=acc[:, q:q + 1],
                        )
            nc.sync.dma_start(yp[:, a:b], yp_d[:, a:b])
            if ci in E2_SLOT:
                # small rotating tiles for the taper (deep WAR-free window),
                # big ones for the 1920 mid-chunks
                if fs <= 768:
                    d = sm_pool.tile([P, 768], f32, tag="ds")
                else:
                    d = mid_pool.tile([P, FS_MAX], f32, tag="d")
                d_tiles[ci] = d
                nc.gpsimd.tensor_sub(d[:, :fs], yt[:, a:b], yp[:, a:b])
            nc.vector._custom_dve(
                MASK1L, out=j_m1[:, :fs], in0=yt[:, a:b], in1=yp[:, a:b],
                s0=THR1, s1=LAM1,
                accum_out=acc[:, ci:ci + 1],
            )
            if ci in R_SET:
                s = U_SLOT[ci]
                for k, thr in enumerate((THR2, THR3)):
                    nc.vector._custom_dve(
                        UABS, out=j_u[:, :fs], in0=yt[:, a:b], in1=yp[:, a:b],
                        s0=thr, s1=0.0,
                        accum_out=acc[:, s + k:s + k + 1],
                    )
        if NCH - 1 in E2_SLOT:
            emit_m2(NCH - 1)

        nc.sync.dma_start(out_d[:], acc[:])

    nc.compile()
    _STATE["nc"] = nc
    return nc


def _run_device(y_pred: np.ndarray, y_true: np.ndarray, **kw):
    nc = _build()
    y_pred = np.asarray(y_pred, dtype=np.float32).reshape(B, -1)
    y_true = np.asarray(y_true, dtype=np.float32).reshape(B, -1)
    in_maps = []
    for c in range(N_CORES):
        sl = slice(c * SHARD_B, (c + 1) * SHARD_B)
        in_maps.append({
            "y_true": np.ascontiguousarray(y_true[sl]).reshape(P, F),
            "y_pred": np.ascontiguousarray(y_pred[sl]).reshape(P, F),
        })
    return run_bass_kernel_spmd(nc, in_maps, list(range(N_CORES)), **kw)


def _finalize(results) -> np.ndarray:
    e1 = e2 = u2 = u3 = 0.0
    cnt = [0.0, 0.0, 0.0]
    for c in range(N_CORES):
        part = results[c]["partials"].astype(np.float64)
        e1 += part[:, 0:NE1].sum()
        e2 += part[:, NE1:NU_BASE].sum()
        for ci in R_SET:
            s = U_SLOT[ci]
            u2 += part[:, s].sum()
            u3 += part[:, s + 1].sum()
        for j in range(len(SD3)):
            cnt[2] += part[:, ND3_BASE + j].sum()
        for j in range(len(SDCC)):
            for t in range(2):
                cnt[t] += part[:, NDCC_BASE + 2 * j + t].sum()
        for g, (sa, sb) in enumerate(SA):
            n_el = P * (sb - sa)
            for t in range(2):
                # ACT slots hold sum(sign): count_ge = (sum(sign)+n_el)/2
                cnt[t] += (part[:, NACC_BASE + 2 * g + t].sum() + n_el) / 2.0
        for g, (sa, sb) in enumerate(SA3):
            n_el = P * (sb - sa)
            cnt[2] += (part[:, NA3_BASE + g].sum() + n_el) / 2.0
    sum_wad = DW1 * e1 + DW2 * e2 + DW2 * u2 + DW3 * u3
    sum_w = W_BASE * N_TOTAL + DW1 * cnt[0] + DW2 * cnt[1] + DW3 * cnt[2]
    return np.array(sum_wad / sum_w, dtype=np.float32)


def kernel(y_pred: np.ndarray, y_true: np.ndarray) -> np.ndarray:
    try:
        res = _run_device(y_pred, y_true)
    except Exception:
        # transient device-state failures have been observed; retry once
        import time as _time
        _time.sleep(2.0)
        res = _run_device(y_pred, y_true)
    return _finalize(res.results)


# revision 17
# speedup vs baseline: 1.0359x; 1.0359x over previous
"""Weighted-MAE loss (nn_MAELoss) on 8 Trainium2 NeuronCores.

reference:  w = bucket-weights(y_true) via thresholds log1p(5/25/50),
            loss = sum(w * |y_true - y_pred|) / sum(w)

Strategy: data-parallel over the batch dim (8 shards of 8 batches); each
core reduces its [128, 15360] shard to per-partition fp32 accumulators;
the host combines them in float64 and divides.

Per-core dataflow (~43.7us DMA floor for 15.7MB/core at 360 GB/s):
  DMA : yt/yp resident in SBUF, streamed in column chunks (yt_c then
        yp_c per chunk; head chunks sized >=480 so the HWDGE 625ns/DMA
        issue rate never starves the DMA engines; tail chunks small to
        shorten the post-stream dependency chain).
  DVE : E1_c = sum(((yt>=T1) + 0.2/29.8) * |yt-yp|)   (diff fused in)
        E2_c = sum(((yt>=T2) + r*(yt>=T3)) * |d|), d from GPSIMD
        tail chunks (R set) avoid the GPSIMD chain entirely:
        U2_c/U3_c = sum((yt>=T2/3) * |yt-yp|) fused ops, plus exact
        is_ge counts (tensor_scalar, 2x perf mode) for late columns.
  ACT : sign-counts for T1/T2/T3 on early columns (bias one ulp below
        each threshold so exact hits count as >=, like the reference).
  Pool: d = yt - yp for mid-stream chunks only (its ~2ns/col rate and
        the +900ns DMA-sem latency make it unusable near stream end).
All engine busy totals sit ~4us under the 43.7us DMA stream so the end
is bounded by the stream + a short tail; host combines in float64.
"""

import os
import sys

import numpy as np

try:
    import concourse  # noqa: F401
except ImportError:  # pragma: no cover
    for _p in ("/root/.axon_site/_ro/trn_rl_repo", "/opt/trn_rl_repo"):
        if os.path.isdir(_p) and _p not in sys.path:
            sys.path.append(_p)

from contextlib import ExitStack
from operator import add

import concourse.bacc as bacc
import concourse.tile as tile
from concourse import mybir
from concourse.bass_utils import run_bass_kernel_spmd
import concourse.dve_ops as dve_ops
from concourse.dve_ops import DveOp
from concourse.dve_spec import (
    C0,
    C1,
    C2,
    Spec,
    Src0,
    Src1,
    Zero,
    _has_src1,
    lower,
    maxx,
)
from concourse.dve_uop import DveOpSpec

# ----------------------------------------------------------------- problem
N_CORES = 8
B, C, T, H, W = 64, 1, 15, 128, 128
SHARD_B = B // N_CORES
P = 128
F = SHARD_B * C * T * H * W // P  # 15360
N_TOTAL = B * C * T * H * W      # 15728640

THR1 = float(np.float32(np.log1p(5.0)))
THR2 = float(np.float32(np.log1p(25.0)))
THR3 = float(np.float32(np.log1p(50.0)))
THRS = (THR1, THR2, THR3)
W_BASE = 0.2          # bucket-0 weight
DW1 = 29.8            # 30 - 0.2
DW2 = 2470.0          # 2500 - 30
DW3 = 17500.0         # 20000 - 2500
LAM1 = float(np.float32(W_BASE / DW1))   # folds 0.2*sum|d| into E1
RATIO32 = float(np.float32(DW3 / DW2))   # folds the T3 level into E2

# ------------------------------------------------------------ span layout
# DMA chunks (shared grid for yt/yp, issued yt_c then yp_c). Tile rotates
# HWDGE DMAs over an 8-slot sem ring: DMA k's issue waits for DMA k-8's
# completion sem (+900ns), so any 7 consecutive transfers must cover the
# ~2.2us issue+DGE path or the stream gaps -> tail chunks taper but stay
# >=256 cols; only the final chunk (fused, post-stream) is smaller.
CHUNKS = [448, 896, 1472] + [1920] * 5 + [768, 608, 480, 384, 320, 256, 128]
assert sum(CHUNKS) == F
NCH = len(CHUNKS)
CUM = [0]
for c in CHUNKS:
    CUM.append(CUM[-1] + c)

# Chunks whose E2-part uses the fused U2/U3 ops (no GPSIMD d): the chunks
# arriving too close to stream end for the Pool sub -> MASK2 chain.
R_SET = {NCH - 1}
POOL_SET = [ci for ci in range(NCH) if ci not in R_SET]

def _rdy(end):
    for i in range(NCH):
        if CUM[i + 1] >= end:
            return i
    raise AssertionError

# Engine rate budget per arriving column is ~2.84ns (360 GB/s, 8B/col);
# every engine's assigned rate must stay below it in each region or the
# deficit spills past the end of the DMA stream:
#   DVE: M1L+M2 2.08 + C3-at-2x 0.52 = 2.60
#   ACT: C1+C2 1.67 + ~0.4 per-op   = 2.1
#   Pool: sub 1.98
# C1/C2 sign spans (ACT, 2 passes): chunk-group spans (cheap per-op
# early); the last CC_TAIL chunks go to DVE tensor_scalar at 2x (ACT's
# 372ns/op fixed cost is too slow once data lands at stream end).
# C3: one early span on ACT (balance knob), per-chunk DVE 2x after.
CC_TAIL = 3          # final chunks whose C1/C2 ride DVE
C3_SPLIT_CH = 2      # C3 on ACT through this chunk index (inclusive)
ACT_GROUPS = [(0, 0), (1, 1), (2, 2), (3, 4), (5, 6),
              (7, 7), (8, 9), (10, 11)]

def _ck(i, j):
    return (CUM[i], CUM[j + 1])

SA = [_ck(i, j) for i, j in ACT_GROUPS if j < NCH - CC_TAIL]
assert SA[-1][1] == CUM[NCH - CC_TAIL]
SA3 = [(0, CUM[C3_SPLIT_CH + 1])]
SD3 = [_ck(i, i) for i in range(C3_SPLIT_CH + 1, NCH)]
SDCC = [_ck(i, i) for i in range(NCH - CC_TAIL, NCH)]

SA_RDY = [_rdy(b) for _, b in SA]
SA3_RDY = [_rdy(b) for _, b in SA3]
SD3_RDY = [_rdy(b) for _, b in SD3]
SDCC_RDY = [_rdy(b) for _, b in SDCC]

# accumulator slot layout in the [P, 128] acc tile
NE1 = NCH                      # E1 per chunk: slots 0..NCH-1
N_HALVES = {ci: (2 if CHUNKS[ci] > 1024 else 1) for ci in POOL_SET}
E2_SLOT = {}
_off = NE1
for ci in POOL_SET:
    E2_SLOT[ci] = _off
    _off += N_HALVES[ci]
NU_BASE = _off
U_SLOT = {ci: NU_BASE + 2 * k for k, ci in enumerate(sorted(R_SET))}
ND3_BASE = NU_BASE + 2 * len(R_SET)      # DVE C3: + j
NDCC_BASE = ND3_BASE + len(SD3)          # DVE C1/C2 tails: + 2*j + k
NACC_BASE = NDCC_BASE + 2 * len(SDCC)    # ACT C1/C2: + 2*g + k
NA3_BASE = NACC_BASE + 2 * len(SA)       # ACT C3: + g
NSLOT = NA3_BASE + len(SA3)
ACC_W = 128
assert NSLOT <= ACC_W

# ------------------------------------------------------- custom DVE ops
_absdiff = maxx(Src0 - Src1, Src1 - Src0)  # |in0 - in1|
_neg = Zero - Src1
_absd = maxx(Src1, _neg)                   # |in1|  (in1 = precomputed d)


def _accum_ref(body_fn):
    def _r(in0, in1, s0, s1, imm2):
        b = body_fn(
            in0.astype(np.float32), None if in1 is None else in1.astype(np.float32),
            s0, s1, imm2,
        ).astype(np.float32)
        return b, b.reshape(b.shape[0], -1).sum(axis=-1, keepdims=True).astype(np.float32)
    return _r


def _register_op(name: str, spec: Spec) -> DveOp:
    for op in dve_ops.OPS:
        if op.name == name:
            return op
    row = dve_ops._CUSTOM_DVE_ROW_BASE + len(dve_ops.OPS)
    assert row < 0x20, "custom-DVE row overflow"
    shas = {}
    for ver in ("v3", "v4"):
        try:
            tmp = DveOpSpec(
                name=name, opcode=row, uops=lower(spec, ver=ver),
                rd1_en=_has_src1(spec),
            )
            shas[ver] = tmp.sha(ver)
        except Exception:
            pass
    op = DveOp(name, spec, subdim=False, uops_sha=shas)
    dve_ops.OPS.append(op)
    dve_ops._SUB_OPCODE_FOR_NAME[name] = row
    dve_ops.CUSTOM_DVE_SPECS[name] = spec
    return op


# out = ((in0 >= s0) + s1) * |in0 - in1| ; accum_out = sum(out)
MASK1L = _register_op(
    "WMAE_MASK1LD_ANT",
    Spec(body=((Src0 >= C0) + C1) * _absdiff, accum=add, accum_init=Zero,
         reference=_accum_ref(
             lambda a, b, s0, s1, i2: ((a >= s0) + s1) * np.abs(a - b))),
)
# out = ((in0 >= s0) + imm2*(in0 >= s1)) * |in1| ; accum_out = sum(out)
MASK2 = _register_op(
    "WMAE_MASK2_ANT",
    Spec(body=((Src0 >= C0) + C2 * (Src0 >= C1)) * _absd,
         accum=add, accum_init=Zero,
         reference=_accum_ref(
             lambda a, b, s0, s1, i2: ((a >= s0) + i2 * (a >= s1)) * np.abs(b))),
)
# out = (in0 >= s0) * |in0 - in1| ; accum_out = sum(out)  (tail chunks)
UABS = _register_op(
    "WMAE_UABS_ANT",
    Spec(body=(Src0 >= C0) * _absdiff, accum=add, accum_init=Zero,
         reference=_accum_ref(
             lambda a, b, s0, s1, i2: (a >= s0) * np.abs(a - b))),
)

_STATE: dict = {}


def _build():
    """Build + schedule the Bass module once per process."""
    if "nc" in _STATE:
        return _STATE["nc"]
    f32 = mybir.dt.float32
    nc = bacc.Bacc("TRN2", target_bir_lowering=False, debug=False,
                   enable_asserts=False)
    yt_d = nc.dram_tensor("y_true", [P, F], f32, kind="ExternalInput").ap()
    yp_d = nc.dram_tensor("y_pred", [P, F], f32, kind="ExternalInput").ap()
    out_d = nc.dram_tensor("partials", [P, ACC_W], f32,
                           kind="ExternalOutput").ap()

    with tile.TileContext(nc) as tc, ExitStack() as ctx:
        big_pool = ctx.enter_context(tc.tile_pool(name="big", bufs=1))
        mid_pool = ctx.enter_context(tc.tile_pool(name="mid", bufs=3))
        sm_pool = ctx.enter_context(tc.tile_pool(name="sm", bufs=6))
        junk_pool = ctx.enter_context(tc.tile_pool(name="junk", bufs=1))
        acc_pool = ctx.enter_context(tc.tile_pool(name="acc", bufs=1))

        yt = big_pool.tile([P, F], f32, tag="yt")
        yp = big_pool.tile([P, F], f32, tag="yp")
        acc = acc_pool.tile([P, ACC_W], f32, tag="acc")

        # sign(y + bias) counts y >= THR; bias = -(one ulp below THR) so an
        # exact threshold hit counts high, matching the reference's y < THR
        def _below(t):
            return float(np.nextafter(np.float32(t), np.float32(0.0)))

        biases = []
        for k, t in enumerate(THRS):
            bt = acc_pool.tile([P, 1], f32, tag=f"bias{k}")
            nc.vector.memset(bt[:], -_below(t))
            biases.append(bt)

        FS_MAX = max(CHUNKS)
        j_m1 = junk_pool.tile([P, FS_MAX], f32, tag="j_m1")
        j_m2 = junk_pool.tile([P, FS_MAX], f32, tag="j_m2")
        j_u = junk_pool.tile([P, 256], f32, tag="j_u")
        DW_MAX = max(b - a for a, b in SD3 + SDCC)
        AW_MAX = max(b - a for a, b in SA + SA3)
        j_cd = junk_pool.tile([P, DW_MAX], f32, tag="j_cd")
        j_ca = junk_pool.tile([P, AW_MAX], f32, tag="j_ca")

        # 1-element dummy Sign pulls the ACT table load into the DMA fill
        nc.scalar.activation(j_u[:, 0:1], biases[0][:],
                             mybir.ActivationFunctionType.Sign,
                             bias=biases[1][:])

        d_tiles: dict = {}

        # Mid-chunk subs and their MASK2 consumers are split in halves so
        # the first M2 half is ready ~2us after yp lands instead of ~3.9us
        # (pool sub is 1.98ns/col); keeps the DVE queue from head-blocking.
        def _halves(ci):
            a, b = CUM[ci], CUM[ci + 1]
            if b - a > 1024:
                m = (a + b) // 2
                return [(a, m), (m, b)]
            return [(a, b)]

        def emit_m2(ci):
            a0 = CUM[ci]
            s = E2_SLOT[ci]
            d = d_tiles.pop(ci)
            for h, (a, b) in enumerate(_halves(ci)):
                nc.vector._custom_dve(
                    MASK2, out=j_m2[:, :b - a], in0=yt[:, a:b],
                    in1=d[:, a - a0:b - a0],
                    s0=THR2, s1=THR3, imm2=RATIO32,
                    accum_out=acc[:, s + h:s + h + 1],
                )

        def emit_counts(ci):
            for g, (sa, sb) in enumerate(SA):
                if SA_RDY[g] == ci:
                    for t in range(2):
                        q = NACC_BASE + 2 * g + t
                        nc.scalar.activation(
                            j_ca[:, :sb - sa], yt[:, sa:sb],
                            mybir.ActivationFunctionType.Sign,
                            bias=biases[t][:],
                            accum_out=acc[:, q:q + 1],
                        )
            for g, (sa, sb) in enumerate(SA3):
                if SA3_RDY[g] == ci:
                    q = NA3_BASE + g
                    nc.scalar.activation(
                        j_ca[:, :sb - sa], yt[:, sa:sb],
                        mybir.ActivationFunctionType.Sign,
                        bias=biases[2][:],
                        accum_out=acc[:, q:q + 1],
                    )
            for j, (sa, sb) in enumerate(SD3):
                if SD3_RDY[j] == ci:
                    q = ND3_BASE + j
                    nc.vector.tensor_scalar(
                        j_cd[:, :sb - sa], yt[:, sa:sb], THR3, 0.0,
                        mybir.AluOpType.is_ge, mybir.AluOpType.add,
                        accum_out=acc[:, q:q + 1],
                    )
            for j, (sa, sb) in enumerate(SDCC):
                if SDCC_RDY[j] == ci:
                    for t in range(2):
                        q = NDCC_BASE + 2 * j + t
                        nc.vector.tensor_scalar(
                            j_cd[:, :sb - sa], yt[:, sa:sb], THRS[t], 0.0,
                            mybir.AluOpType.is_ge, mybir.AluOpType.add,
                            accum_out=acc[:, q:q + 1],
                        )

        def emit_sub(ci):
            a0, b0 = CUM[ci], CUM[ci + 1]
            if b0 - a0 <= 768:
                d = sm_pool.tile([P, 768], f32, tag="ds")
            else:
                d = mid_pool.tile([P, FS_MAX], f32, tag="d")
            d_tiles[ci] = d
            for a, b in _halves(ci):
                nc.gpsimd.tensor_sub(d[:, a - a0:b - a0], yt[:, a:b],
                                     yp[:, a:b])

        def emit_prods(ci):
            a, b = CUM[ci], CUM[ci + 1]
            fs = b - a
            nc.vector._custom_dve(
                MASK1L, out=j_m1[:, :fs], in0=yt[:, a:b], in1=yp[:, a:b],
                s0=THR1, s1=LAM1,
                accum_out=acc[:, ci:ci + 1],
            )
            if ci in R_SET:
                s = U_SLOT[ci]
                for k, thr in enumerate((THR2, THR3)):
                    nc.vector._custom_dve(
                        UABS, out=j_u[:, :fs], in0=yt[:, a:b], in1=yp[:, a:b],
                        s0=thr, s1=0.0,
                        accum_out=acc[:, s + k:s + k + 1],
                    )

        # Last two chunks: both yt DMAs go before both yp DMAs so the
        # yt-only count ops clear the DVE queue during the final transfers.
        for ci in range(NCH - 2):
            a, b = CUM[ci], CUM[ci + 1]
            nc.sync.dma_start(yt[:, a:b], yt_d[:, a:b])
            emit_counts(ci)
            if ci - 1 in E2_SLOT:
                emit_m2(ci - 1)
            nc.sync.dma_start(yp[:, a:b], yp_d[:, a:b])
            if ci in E2_SLOT:
                emit_sub(ci)
            emit_prods(ci)
        cp, cq = NCH - 2, NCH - 1
        for ci in (cp, cq):
            a, b = CUM[ci], CUM[ci + 1]
            nc.sync.dma_start(yt[:, a:b], yt_d[:, a:b])
            emit_counts(ci)
        if cp - 1 in E2_SLOT:
            emit_m2(cp - 1)
        for ci in (cp, cq):
            a, b = CUM[ci], CUM[ci + 1]
            nc.sync.dma_start(yp[:, a:b], yp_d[:, a:b])
            if ci in E2_SLOT:
                emit_sub(ci)
            emit_prods(ci)
        # the last pool chunk's M2 lands after the final products: its d is
        # only ready ~1us past the last yp (sub chain), so it goes last.
        for ci in (cp, cq):
            if ci in E2_SLOT and ci in d_tiles:
                emit_m2(ci)

        nc.sync.dma_start(out_d[:], acc[:])

    nc.compile()
    _STATE["nc"] = nc
    return nc


def _run_device(y_pred: np.ndarray, y_true: np.ndarray, **kw):
    nc = _build()
    y_pred = np.asarray(y_pred, dtype=np.float32).reshape(B, -1)
    y_true = np.asarray(y_true, dtype=np.float32).reshape(B, -1)
    in_maps = []
    for c in range(N_CORES):
        sl = slice(c * SHARD_B, (c + 1) * SHARD_B)
        in_maps.append({
            "y_true": np.ascontiguousarray(y_true[sl]).reshape(P, F),
            "y_pred": np.ascontiguousarray(y_pred[sl]).reshape(P, F),
        })
    return run_bass_kernel_spmd(nc, in_maps, list(range(N_CORES)), **kw)


def _finalize(results) -> np.ndarray:
    e1 = e2 = u2 = u3 = 0.0
    cnt = [0.0, 0.0, 0.0]
    for c in range(N_CORES):
        part = results[c]["partials"].astype(np.float64)
        e1 += part[:, 0:NE1].sum()
        e2 += part[:, NE1:NU_BASE].sum()
        for ci in R_SET:
            s = U_SLOT[ci]
            u2 += part[:, s].sum()
            u3 += part[:, s + 1].sum()
        for j in range(len(SD3)):
            cnt[2] += part[:, ND3_BASE + j].sum()
        for j in range(len(SDCC)):
            for t in range(2):
                cnt[t] += part[:, NDCC_BASE + 2 * j + t].sum()
        for g, (sa, sb) in enumerate(SA):
            n_el = P * (sb - sa)
            for t in range(2):
                # ACT slots hold sum(sign): count_ge = (sum(sign)+n_el)/2
                cnt[t] += (part[:, NACC_BASE + 2 * g + t].sum() + n_el) / 2.0
        for g, (sa, sb) in enumerate(SA3):
            n_el = P * (sb - sa)
            cnt[2] += (part[:, NA3_BASE + g].sum() + n_el) / 2.0
    sum_wad = DW1 * e1 + DW2 * e2 + DW2 * u2 + DW3 * u3
    sum_w = W_BASE * N_TOTAL + DW1 * cnt[0] + DW2 * cnt[1] + DW3 * cnt[2]
    return np.array(sum_wad / sum_w, dtype=np.float32)


def kernel(y_pred: np.ndarray, y_true: np.ndarray) -> np.ndarray:
    try:
        res = _run_device(y_pred, y_true)
    except Exception:
        # transient device-state failures have been observed; retry once
        import time as _time
        _time.sleep(2.0)
        res = _run_device(y_pred, y_true)
    return _finalize(res.results)


# revision 19
# speedup vs baseline: 1.0372x; 1.0012x over previous
"""Weighted-MAE loss (nn_MAELoss) on 8 Trainium2 NeuronCores.

reference:  w = bucket-weights(y_true) via thresholds log1p(5/25/50),
            loss = sum(w * |y_true - y_pred|) / sum(w)

Strategy: data-parallel over the batch dim (8 shards of 8 batches); each
core reduces its [128, 15360] shard to per-partition fp32 accumulators;
the host combines them in float64 and divides.

Per-core dataflow (~43.7us DMA floor for 15.7MB/core at 360 GB/s):
  DMA : yt/yp resident in SBUF, streamed in column chunks (yt_c then
        yp_c per chunk; head chunks sized >=480 so the HWDGE 625ns/DMA
        issue rate never starves the DMA engines; tail chunks small to
        shorten the post-stream dependency chain).
  DVE : E1_c = sum(((yt>=T1) + 0.2/29.8) * |yt-yp|)   (diff fused in)
        E2_c = sum(((yt>=T2) + r*(yt>=T3)) * |d|), d from GPSIMD
        tail chunks (R set) avoid the GPSIMD chain entirely:
        U2_c/U3_c = sum((yt>=T2/3) * |yt-yp|) fused ops, plus exact
        is_ge counts (tensor_scalar, 2x perf mode) for late columns.
  ACT : sign-counts for T1/T2/T3 on early columns (bias one ulp below
        each threshold so exact hits count as >=, like the reference).
  Pool: d = yt - yp for mid-stream chunks only (its ~2ns/col rate and
        the +900ns DMA-sem latency make it unusable near stream end).
All engine busy totals sit ~4us under the 43.7us DMA stream so the end
is bounded by the stream + a short tail; host combines in float64.
"""

import os
import sys

import numpy as np

try:
    import concourse  # noqa: F401
except ImportError:  # pragma: no cover
    for _p in ("/root/.axon_site/_ro/trn_rl_repo", "/opt/trn_rl_repo"):
        if os.path.isdir(_p) and _p not in sys.path:
            sys.path.append(_p)

from contextlib import ExitStack
from operator import add

import concourse.bacc as bacc
import concourse.tile as tile
from concourse import mybir
from concourse.bass_utils import run_bass_kernel_spmd
import concourse.dve_ops as dve_ops
from concourse.dve_ops import DveOp
from concourse.dve_spec import (
    C0,
    C1,
    C2,
    Spec,
    Src0,
    Src1,
    Zero,
    _has_src1,
    lower,
    maxx,
)
from concourse.dve_uop import DveOpSpec

# ----------------------------------------------------------------- problem
N_CORES = 8
B, C, T, H, W = 64, 1, 15, 128, 128
SHARD_B = B // N_CORES
P = 128
F = SHARD_B * C * T * H * W // P  # 15360
N_TOTAL = B * C * T * H * W      # 15728640

THR1 = float(np.float32(np.log1p(5.0)))
THR2 = float(np.float32(np.log1p(25.0)))
THR3 = float(np.float32(np.log1p(50.0)))
THRS = (THR1, THR2, THR3)
W_BASE = 0.2          # bucket-0 weight
DW1 = 29.8            # 30 - 0.2
DW2 = 2470.0          # 2500 - 30
DW3 = 17500.0         # 20000 - 2500
LAM1 = float(np.float32(W_BASE / DW1))   # folds 0.2*sum|d| into E1
RATIO32 = float(np.float32(DW3 / DW2))   # folds the T3 level into E2

# ------------------------------------------------------------ span layout
# DMA chunks (shared grid for yt/yp, issued yt_c then yp_c). Tile rotates
# HWDGE DMAs over an 8-slot sem ring: DMA k's issue waits for DMA k-8's
# completion sem (+900ns), so any 7 consecutive transfers must cover the
# ~2.2us issue+DGE path or the stream gaps -> tail chunks taper but stay
# >=256 cols; only the final chunk (fused, post-stream) is smaller.
CHUNKS = [448, 896, 1472] + [1920] * 5 + [768, 608, 480, 384, 320, 256, 128]
assert sum(CHUNKS) == F
NCH = len(CHUNKS)
CUM = [0]
for c in CHUNKS:
    CUM.append(CUM[-1] + c)

# Chunks whose E2-part uses the fused U2/U3 ops (no GPSIMD d): the chunks
# arriving too close to stream end for the Pool sub -> MASK2 chain.
R_SET = {NCH - 1}
POOL_SET = [ci for ci in range(NCH) if ci not in R_SET]

def _rdy(end):
    for i in range(NCH):
        if CUM[i + 1] >= end:
            return i
    raise AssertionError

# Engine rate budget per arriving column is ~2.84ns (360 GB/s, 8B/col);
# every engine's assigned rate must stay below it in each region or the
# deficit spills past the end of the DMA stream:
#   DVE: M1L+M2 2.08 + C3-at-2x 0.52 = 2.60
#   ACT: C1+C2 1.67 + ~0.4 per-op   = 2.1
#   Pool: sub 1.98
# C1/C2 sign spans (ACT, 2 passes): chunk-group spans (cheap per-op
# early); the last CC_TAIL chunks go to DVE tensor_scalar at 2x (ACT's
# 372ns/op fixed cost is too slow once data lands at stream end).
# C3: one early span on ACT (balance knob), per-chunk DVE 2x after.
CC_TAIL = 2          # final chunks whose C1/C2 ride DVE
C3_SPLIT_CH = 3      # C3 on ACT through this chunk index (inclusive)
ACT_GROUPS = [(0, 0), (1, 1), (2, 2), (3, 4), (5, 6),
              (7, 7), (8, 9), (10, 11), (12, 12)]

def _ck(i, j):
    return (CUM[i], CUM[j + 1])

SA = [_ck(i, j) for i, j in ACT_GROUPS if j < NCH - CC_TAIL]
assert SA[-1][1] == CUM[NCH - CC_TAIL]
SA3 = [(0, CUM[3]), (CUM[3], CUM[C3_SPLIT_CH + 1])] if C3_SPLIT_CH >= 3 else [(0, CUM[C3_SPLIT_CH + 1])]
SD3 = [_ck(i, i) for i in range(C3_SPLIT_CH + 1, NCH)]
SDCC = [_ck(i, i) for i in range(NCH - CC_TAIL, NCH)]

SA_RDY = [_rdy(b) for _, b in SA]
SA3_RDY = [_rdy(b) for _, b in SA3]
SD3_RDY = [_rdy(b) for _, b in SD3]
SDCC_RDY = [_rdy(b) for _, b in SDCC]

# accumulator slot layout in the [P, 128] acc tile
NE1 = NCH                      # E1 per chunk: slots 0..NCH-1
N_HALVES = {ci: (2 if CHUNKS[ci] > 1024 else 1) for ci in POOL_SET}
E2_SLOT = {}
_off = NE1
for ci in POOL_SET:
    E2_SLOT[ci] = _off
    _off += N_HALVES[ci]
NU_BASE = _off
U_SLOT = {ci: NU_BASE + 2 * k for k, ci in enumerate(sorted(R_SET))}
ND3_BASE = NU_BASE + 2 * len(R_SET)      # DVE C3: + j
NDCC_BASE = ND3_BASE + len(SD3)          # DVE C1/C2 tails: + 2*j + k
NACC_BASE = NDCC_BASE + 2 * len(SDCC)    # ACT C1/C2: + 2*g + k
NA3_BASE = NACC_BASE + 2 * len(SA)       # ACT C3: + g
NSLOT = NA3_BASE + len(SA3)
ACC_W = 128
assert NSLOT <= ACC_W

# ------------------------------------------------------- custom DVE ops
_absdiff = maxx(Src0 - Src1, Src1 - Src0)  # |in0 - in1|
_neg = Zero - Src1
_absd = maxx(Src1, _neg)                   # |in1|  (in1 = precomputed d)


def _accum_ref(body_fn):
    def _r(in0, in1, s0, s1, imm2):
        b = body_fn(
            in0.astype(np.float32), None if in1 is None else in1.astype(np.float32),
            s0, s1, imm2,
        ).astype(np.float32)
        return b, b.reshape(b.shape[0], -1).sum(axis=-1, keepdims=True).astype(np.float32)
    return _r


def _register_op(name: str, spec: Spec) -> DveOp:
    for op in dve_ops.OPS:
        if op.name == name:
            return op
    row = dve_ops._CUSTOM_DVE_ROW_BASE + len(dve_ops.OPS)
    assert row < 0x20, "custom-DVE row overflow"
    shas = {}
    for ver in ("v3", "v4"):
        try:
            tmp = DveOpSpec(
                name=name, opcode=row, uops=lower(spec, ver=ver),
                rd1_en=_has_src1(spec),
            )
            shas[ver] = tmp.sha(ver)
        except Exception:
            pass
    op = DveOp(name, spec, subdim=False, uops_sha=shas)
    dve_ops.OPS.append(op)
    dve_ops._SUB_OPCODE_FOR_NAME[name] = row
    dve_ops.CUSTOM_DVE_SPECS[name] = spec
    return op


# out = ((in0 >= s0) + s1) * |in0 - in1| ; accum_out = sum(out)
MASK1L = _register_op(
    "WMAE_MASK1LD_ANT",
    Spec(body=((Src0 >= C0) + C1) * _absdiff, accum=add, accum_init=Zero,
         reference=_accum_ref(
             lambda a, b, s0, s1, i2: ((a >= s0) + s1) * np.abs(a - b))),
)
# out = ((in0 >= s0) + imm2*(in0 >= s1)) * |in1| ; accum_out = sum(out)
MASK2 = _register_op(
    "WMAE_MASK2_ANT",
    Spec(body=((Src0 >= C0) + C2 * (Src0 >= C1)) * _absd,
         accum=add, accum_init=Zero,
         reference=_accum_ref(
             lambda a, b, s0, s1, i2: ((a >= s0) + i2 * (a >= s1)) * np.abs(b))),
)
# out = (in0 >= s0) * |in0 - in1| ; accum_out = sum(out)  (tail chunks)
UABS = _register_op(
    "WMAE_UABS_ANT",
    Spec(body=(Src0 >= C0) * _absdiff, accum=add, accum_init=Zero,
         reference=_accum_ref(
             lambda a, b, s0, s1, i2: (a >= s0) * np.abs(a - b))),
)

_STATE: dict = {}


def _build():
    """Build + schedule the Bass module once per process."""
    if "nc" in _STATE:
        return _STATE["nc"]
    f32 = mybir.dt.float32
    nc = bacc.Bacc("TRN2", target_bir_lowering=False, debug=False,
                   enable_asserts=False)
    yt_d = nc.dram_tensor("y_true", [P, F], f32, kind="ExternalInput").ap()
    yp_d = nc.dram_tensor("y_pred", [P, F], f32, kind="ExternalInput").ap()
    out_d = nc.dram_tensor("partials", [P, ACC_W], f32,
                           kind="ExternalOutput").ap()

    with tile.TileContext(nc) as tc, ExitStack() as ctx:
        big_pool = ctx.enter_context(tc.tile_pool(name="big", bufs=1))
        mid_pool = ctx.enter_context(tc.tile_pool(name="mid", bufs=2))
        sm_pool = ctx.enter_context(tc.tile_pool(name="sm", bufs=8))
        junk_pool = ctx.enter_context(tc.tile_pool(name="junk", bufs=1))
        acc_pool = ctx.enter_context(tc.tile_pool(name="acc", bufs=1))

        yt = big_pool.tile([P, F], f32, tag="yt")
        yp = big_pool.tile([P, F], f32, tag="yp")
        acc = acc_pool.tile([P, ACC_W], f32, tag="acc")

        # sign(y + bias) counts y >= THR; bias = -(one ulp below THR) so an
        # exact threshold hit counts high, matching the reference's y < THR
        def _below(t):
            return float(np.nextafter(np.float32(t), np.float32(0.0)))

        biases = []
        for k, t in enumerate(THRS):
            bt = acc_pool.tile([P, 1], f32, tag=f"bias{k}")
            nc.vector.memset(bt[:], -_below(t))
            biases.append(bt)

        FS_MAX = max(CHUNKS)
        j_m1 = junk_pool.tile([P, FS_MAX], f32, tag="j_m1")
        j_m2 = j_m1
        j_u = junk_pool.tile([P, 256], f32, tag="j_u")
        DW_MAX = max(b - a for a, b in SD3 + SDCC)
        AW_MAX = max(b - a for a, b in SA + SA3)
        j_cd = junk_pool.tile([P, DW_MAX], f32, tag="j_cd")
        j_ca = junk_pool.tile([P, AW_MAX], f32, tag="j_ca")

        # 1-element dummy Sign pulls the ACT table load into the DMA fill
        nc.scalar.activation(j_u[:, 0:1], biases[0][:],
                             mybir.ActivationFunctionType.Sign,
                             bias=biases[1][:])

        d_tiles: dict = {}

        # Mid-chunk subs and their MASK2 consumers are split in halves so
        # the first M2 half is ready ~2us after yp lands instead of ~3.9us
        # (pool sub is 1.98ns/col); keeps the DVE queue from head-blocking.
        def _halves(ci):
            a, b = CUM[ci], CUM[ci + 1]
            if b - a > 1024:
                m = (a + b) // 2
                return [(a, m), (m, b)]
            return [(a, b)]

        def emit_m2(ci):
            a0 = CUM[ci]
            s = E2_SLOT[ci]
            d = d_tiles.pop(ci)
            for h, (a, b) in enumerate(_halves(ci)):
                nc.vector._custom_dve(
                    MASK2, out=j_m2[:, :b - a], in0=yt[:, a:b],
                    in1=d[:, a - a0:b - a0],
                    s0=THR2, s1=THR3, imm2=RATIO32,
                    accum_out=acc[:, s + h:s + h + 1],
                )

        def emit_counts(ci):
            for g, (sa, sb) in enumerate(SA):
                if SA_RDY[g] == ci:
                    for t in range(2):
                        q = NACC_BASE + 2 * g + t
                        nc.scalar.activation(
                            j_ca[:, :sb - sa], yt[:, sa:sb],
                            mybir.ActivationFunctionType.Sign,
                            bias=biases[t][:],
                            accum_out=acc[:, q:q + 1],
                        )
            for g, (sa, sb) in enumerate(SA3):
                if SA3_RDY[g] == ci:
                    q = NA3_BASE + g
                    nc.scalar.activation(
                        j_ca[:, :sb - sa], yt[:, sa:sb],
                        mybir.ActivationFunctionType.Sign,
                        bias=biases[2][:],
                        accum_out=acc[:, q:q + 1],
                    )
            for j, (sa, sb) in enumerate(SD3):
                if SD3_RDY[j] == ci:
                    q = ND3_BASE + j
                    nc.vector.tensor_scalar(
                        j_cd[:, :sb - sa], yt[:, sa:sb], THR3, 0.0,
                        mybir.AluOpType.is_ge, mybir.AluOpType.add,
                        accum_out=acc[:, q:q + 1],
                    )
            for j, (sa, sb) in enumerate(SDCC):
                if SDCC_RDY[j] == ci:
                    for t in range(2):
                        q = NDCC_BASE + 2 * j + t
                        nc.vector.tensor_scalar(
                            j_cd[:, :sb - sa], yt[:, sa:sb], THRS[t], 0.0,
                            mybir.AluOpType.is_ge, mybir.AluOpType.add,
                            accum_out=acc[:, q:q + 1],
                        )

        def emit_sub(ci):
            a0, b0 = CUM[ci], CUM[ci + 1]
            if b0 - a0 <= 768:
                d = sm_pool.tile([P, 768], f32, tag="ds")
            else:
                d = mid_pool.tile([P, FS_MAX], f32, tag="d")
            d_tiles[ci] = d
            for a, b in _halves(ci):
                nc.gpsimd.tensor_sub(d[:, a - a0:b - a0], yt[:, a:b],
                                     yp[:, a:b])

        def emit_prods(ci):
            a, b = CUM[ci], CUM[ci + 1]
            fs = b - a
            nc.vector._custom_dve(
                MASK1L, out=j_m1[:, :fs], in0=yt[:, a:b], in1=yp[:, a:b],
                s0=THR1, s1=LAM1,
                accum_out=acc[:, ci:ci + 1],
            )
            if ci in R_SET:
                s = U_SLOT[ci]
                for k, thr in enumerate((THR2, THR3)):
                    nc.vector._custom_dve(
                        UABS, out=j_u[:, :fs], in0=yt[:, a:b], in1=yp[:, a:b],
                        s0=thr, s1=0.0,
                        accum_out=acc[:, s + k:s + k + 1],
                    )

        # Last two chunks: both yt DMAs go before both yp DMAs so the
        # yt-only count ops clear the DVE queue during the final transfers.
        for ci in range(NCH - 2):
            a, b = CUM[ci], CUM[ci + 1]
            nc.sync.dma_start(yt[:, a:b], yt_d[:, a:b])
            emit_counts(ci)
            if ci - 1 in E2_SLOT:
                emit_m2(ci - 1)
            nc.sync.dma_start(yp[:, a:b], yp_d[:, a:b])
            if ci in E2_SLOT:
                emit_sub(ci)
            emit_prods(ci)
        cp, cq = NCH - 2, NCH - 1
        for ci in (cp, cq):
            a, b = CUM[ci], CUM[ci + 1]
            nc.sync.dma_start(yt[:, a:b], yt_d[:, a:b])
            emit_counts(ci)
        if cp - 1 in E2_SLOT:
            emit_m2(cp - 1)
        for ci in (cp, cq):
            a, b = CUM[ci], CUM[ci + 1]
            nc.sync.dma_start(yp[:, a:b], yp_d[:, a:b])
            if ci in E2_SLOT:
                emit_sub(ci)
            emit_prods(ci)
        # the last pool chunk's M2 lands after the final products: its d is
        # only ready ~1us past the last yp (sub chain), so it goes last.
        for ci in (cp, cq):
            if ci in E2_SLOT and ci in d_tiles:
                emit_m2(ci)

        nc.sync.dma_start(out_d[:], acc[:])

    nc.compile()
    _STATE["nc"] = nc
    return nc


def _run_device(y_pred: np.ndarray, y_true: np.ndarray, **kw):
    nc = _build()
    y_pred = np.asarray(y_pred, dtype=np.float32).reshape(B, -1)
    y_true = np.asarray(y_true, dtype=np.float32).reshape(B, -1)
    in_maps = []
    for c in range(N_CORES):
        sl = slice(c * SHARD_B, (c + 1) * SHARD_B)
        in_maps.append({
            "y_true": np.ascontiguousarray(y_true[sl]).reshape(P, F),
            "y_pred": np.ascontiguousarray(y_pred[sl]).reshape(P, F),
        })
    return run_bass_kernel_spmd(nc, in_maps, list(range(N_CORES)), **kw)


def _finalize(results) -> np.ndarray:
    e1 = e2 = u2 = u3 = 0.0
    cnt = [0.0, 0.0, 0.0]
    for c in range(N_CORES):
        part = results[c]["partials"].astype(np.float64)
        e1 += part[:, 0:NE1].sum()
        e2 += part[:, NE1:NU_BASE].sum()
        for ci in R_SET:
            s = U_SLOT[ci]
            u2 += part[:, s].sum()
            u3 += part[:, s + 1].sum()
        for j in range(len(SD3)):
            cnt[2] += part[:, ND3_BASE + j].sum()
        for j in range(len(SDCC)):
            for t in range(2):
                cnt[t] += part[:, NDCC_BASE + 2 * j + t].sum()
        for g, (sa, sb) in enumerate(SA):
            n_el = P * (sb - sa)
            for t in range(2):
                # ACT slots hold sum(sign): count_ge = (sum(sign)+n_el)/2
                cnt[t] += (part[:, NACC_BASE + 2 * g + t].sum() + n_el) / 2.0
        for g, (sa, sb) in enumerate(SA3):
            n_el = P * (sb - sa)
            cnt[2] += (part[:, NA3_BASE + g].sum() + n_el) / 2.0
    sum_wad = DW1 * e1 + DW2 * e2 + DW2 * u2 + DW3 * u3
    sum_w = W_BASE * N_TOTAL + DW1 * cnt[0] + DW2 * cnt[1] + DW3 * cnt[2]
    return np.array(sum_wad / sum_w, dtype=np.float32)


def kernel(y_pred: np.ndarray, y_true: np.ndarray) -> np.ndarray:
    try:
        res = _run_device(y_pred, y_true)
    except Exception:
        # transient device-state failures have been observed; retry once
        import time as _time
        _time.sleep(2.0)
        res = _run_device(y_pred, y_true)
    return _finalize(res.results)
